# revision 16
# baseline (speedup 1.0000x reference)
"""Trainium2 Bass kernel for nn_ConvLayer: 3x3 conv (stride 1, pad 1) + per-channel offset.

Problem: x[32,64,56,56] (*) w[128,64,3,3] + offset[128,1,1] -> out[32,128,56,56], fp32.

Strategy (8 NeuronCores, data-parallel over batch, 4 images/core):
  - Conv as 9 shifted matmuls (one per 3x3 tap) accumulated in PSUM.
  - CIN=64 -> each tap is a contract-64 matmul = half the 128x128 PE array.
    Two images are processed CONCURRENTLY via 64x128 row tiling: image A's
    channels live in SBUF partitions 0-63 (PE tile (0,0)), image B's in
    partitions 64-127 (PE tile (64,0)). Each accumulates into its own PSUM
    bank, reaching full PE-array packing with no data duplication.
  - Host pre-pads x to a 58x58 grid (zeros on borders) so every tap is a
    contiguous shifted window; host pre-transposes the weight to [cin,tap,k]
    (lhsT layout) and duplicates it into both partition halves.
  - Output columns are produced on the padded 58-wide grid; the PSUM->SBUF
    eviction (ScalarE for image A, VectorE for image B) compacts back to the
    dense 56-wide grid and fuses the per-channel offset add, so the store DMA
    is fully contiguous.
"""

import numpy as np
from contextlib import ExitStack

import concourse.bass as bass
import concourse.tile as tile
from concourse import bacc, mybir
from concourse.bass_utils import run_bass_kernel_spmd

# Problem constants (hardcoded per contract).
B, CIN, HW, K = 32, 64, 56, 128
NCORES = 8
BPC = B // NCORES          # images per core
HP = HW + 2                # padded row width: 58
NPAD = HP * HP + 4         # padded image + slack for tap reads: 3368
NOUT = HW * HW             # 3136
ROWS_PER_CHUNK = 8
CHUNK = ROWS_PER_CHUNK * HP     # 464 <= 512 (one PSUM bank, fp32)
DCHUNK = ROWS_PER_CHUNK * HW    # 448 dense output cols per chunk
NCHUNKS = HW // ROWS_PER_CHUNK  # 7
TAPS = 9
F32 = mybir.dt.float32
F32R = mybir.dt.float32r

_NC_CACHE = None


def _conv_kernel(ctx: ExitStack, tc: "tile.TileContext", out_ap, xp_ap, w2_ap, off_ap):
    nc = tc.nc
    singles = ctx.enter_context(tc.tile_pool(name="singles", bufs=1))
    xpool = ctx.enter_context(tc.tile_pool(name="xpool", bufs=2))
    opool = ctx.enter_context(tc.tile_pool(name="opool", bufs=2))
    psum = ctx.enter_context(tc.tile_pool(name="psum", bufs=8, space="PSUM"))

    # Chunk groups: first group is a single chunk so its input slice is small
    # and the first matmul starts as early as possible; later groups pair
    # chunks to amortize weight loads. 4 PSUM banks max per group, 8 total
    # with double buffering.
    groups = [(0,), (1, 2), (3, 4), (5,), (6,)]
    # x-load slices: slice g covers every tap read of chunk group g
    # (chunk c reads cols < 464*c + 582), so group g's matmuls gate only on
    # slices <= g.
    xbounds = [0, 584, 1512, 2440, 2904, NPAD]
    # Output store slices: one per chunk group (dense cols).
    obounds = [0, 1 * DCHUNK, 3 * DCHUNK, 5 * DCHUNK, 6 * DCHUNK, NOUT]

    # Weights as lhsT [c, tap, k], duplicated across both partition halves.
    # Split across both HWDGE rings and dispatched first: they gate the very
    # first matmul, and a single-ring 0.59MB load would delay x slices.
    w_sb = singles.tile([128, TAPS, K], F32R)
    nc.sync.dma_start(w_sb[0:64], w2_ap[0:64])
    nc.scalar.dma_start(w_sb[64:128], w2_ap[64:128])
    off_sb = singles.tile([128, 1], F32)
    nc.scalar.dma_start(off_sb[:], off_ap[:])

    # PE warmup: cheap bf16 matmuls on scratch keep TensorE busy during the
    # input-DMA head so the HAM clock gate opens (1.2 -> 2.4 GHz) early.
    # Few enough that they finish before the first input slice lands (the PE
    # queue is FIFO, so excess warmups would delay the real matmuls).
    scratch = singles.tile([128, 512], mybir.dt.bfloat16)
    nc.vector.memset(scratch[:], 0.0)
    ps_warm = psum.tile([128, 512], F32, tag="ps", name="ps_warm")
    for _ in range(7):
        nc.tensor.matmul(
            ps_warm[:], lhsT=scratch[0:64, 0:128], rhs=scratch[0:64, :],
            start=True, stop=True,
        )

    for pair in range(BPC // 2):
        b0 = 2 * pair
        # Both images of the pair side by side: [2, CIN, NPAD] -> [128, NPAD],
        # split into 4 column slices so early chunk groups start ASAP.
        x_t = xpool.tile([128, NPAD], F32R, tag="x")
        xsrc = xp_ap[b0 : b0 + 2].rearrange("b c n -> (b c) n")
        for s in range(len(xbounds) - 1):
            # Alternate the two HWDGE rings so input slices drain in parallel.
            eng = nc.sync if s % 2 == 0 else nc.scalar
            eng.dma_start(
                x_t[:, xbounds[s] : xbounds[s + 1]],
                xsrc[:, xbounds[s] : xbounds[s + 1]],
            )
        o_sb = [
            opool.tile([128, NOUT], F32, tag="oA", name=f"oA_{pair}"),
            opool.tile([128, NOUT], F32, tag="oB", name=f"oB_{pair}"),
        ]

        for g, grp in enumerate(groups):
            ps = {}
            for half in (0, 1):
                for c in grp:
                    ps[(half, c)] = psum.tile(
                        [128, CHUNK], F32, tag="ps", name=f"ps_{pair}_{half}_{c}"
                    )
            for t in range(TAPS):
                kh, kw = divmod(t, 3)
                o = kh * HP + kw
                st, sp = (t == 0), (t == TAPS - 1)
                for half in (0, 1):
                    lo, hi = 64 * half, 64 * half + 64
                    for c in grp:
                        # float32r: single-pass fp32 matmul (full rate at
                        # N>=256) vs plain fp32's two half-rate passes.
                        nc.tensor.matmul(
                            ps[(half, c)][:],
                            lhsT=w_sb[lo:hi, t, :],
                            rhs=x_t[lo:hi, o + CHUNK * c : o + CHUNK * c + CHUNK],
                            start=st,
                            stop=sp,
                        )
            # Evict: compact 58-wide padded rows to 56-wide dense rows and add
            # the per-channel offset. Image A on ScalarE, image B on VectorE.
            for c in grp:
                pa = ps[(0, c)].rearrange("p (r x) -> p r x", x=HP)[:, :, 0:HW]
                oa = o_sb[0][:, c * DCHUNK : (c + 1) * DCHUNK].rearrange(
                    "p (r x) -> p r x", x=HW
                )
                nc.scalar.add(oa, pa, off_sb)
                pb = ps[(1, c)].rearrange("p (r x) -> p r x", x=HP)[:, :, 0:HW]
                ob = o_sb[1][:, c * DCHUNK : (c + 1) * DCHUNK].rearrange(
                    "p (r x) -> p r x", x=HW
                )
                nc.vector.tensor_scalar_add(ob, pb, off_sb)
            # Stream this group's output slice out immediately. Image A rides
            # the Scalar HWDGE ring, image B the Sync ring, so the two output
            # streams (and the input stream) drain in parallel.
            nc.scalar.dma_start(
                out_ap[b0][:, obounds[g] : obounds[g + 1]],
                o_sb[0][:, obounds[g] : obounds[g + 1]],
            )
            nc.sync.dma_start(
                out_ap[b0 + 1][:, obounds[g] : obounds[g + 1]],
                o_sb[1][:, obounds[g] : obounds[g + 1]],
            )


def _build_nc():
    global _NC_CACHE
    if _NC_CACHE is not None:
        return _NC_CACHE
    nc = bacc.Bacc(
        "TRN2", target_bir_lowering=False, debug=False, num_devices=NCORES
    )
    xp_ap = nc.dram_tensor("xp", [BPC, CIN, NPAD], F32R, kind="ExternalInput").ap()
    w2_ap = nc.dram_tensor("w2", [128, TAPS, K], F32R, kind="ExternalInput").ap()
    off_ap = nc.dram_tensor("off", [K, 1], F32, kind="ExternalInput").ap()
    out_ap = nc.dram_tensor("out", [BPC, K, NOUT], F32, kind="ExternalOutput").ap()
    with tile.TileContext(nc) as tc:
        with ExitStack() as ctx:
            _conv_kernel(ctx, tc, out_ap, xp_ap, w2_ap, off_ap)
    nc.compile()
    _NC_CACHE = nc
    return nc


def _prep_inputs(x, weight, offset):
    """Host-side layout prep: pad x, transpose+duplicate weights."""
    x = np.ascontiguousarray(np.asarray(x, dtype=np.float32))
    weight = np.asarray(weight, dtype=np.float32)
    offset = np.asarray(offset, dtype=np.float32)

    xph = np.zeros((B, CIN, NPAD), dtype=np.float32)
    xph[:, :, : HP * HP].reshape(B, CIN, HP, HP)[:, :, 1 : 1 + HW, 1 : 1 + HW] = x

    wt = np.ascontiguousarray(weight.transpose(1, 2, 3, 0)).reshape(CIN, TAPS, K)
    w2 = np.ascontiguousarray(np.concatenate([wt, wt], axis=0))  # [128, 9, 128]
    off = np.ascontiguousarray(offset.reshape(K, 1))
    return xph, w2, off


def kernel(x, weight, offset):
    nc = _build_nc()
    xph, w2, off = _prep_inputs(x, weight, offset)
    in_maps = [
        {"xp": xph[i * BPC : (i + 1) * BPC], "w2": w2, "off": off}
        for i in range(NCORES)
    ]
    res = run_bass_kernel_spmd(nc, in_maps, list(range(NCORES))).results
    out = np.concatenate(
        [res[i]["out"].reshape(BPC, K, HW, HW) for i in range(NCORES)], axis=0
    )
    return out
